# revision 15
# baseline (speedup 1.0000x reference)
"""Trainium2 Bass kernel for nn_CaC_Module (dynamic-kernel dilated depthwise CNN).

Per-sample computation (b=8 sharded 1/core across 8 NeuronCores):
  query = Wq @ x          (1x1 conv, [9, hw])
  q     = softmax(query over hw)          (bq cancels in softmax -> ignored)
  kern  = Wk @ (x @ q^T) + bk             (associativity: avoids the big
                                           key GEMM entirely; bk folds in
                                           because sum_n q = 1)
  out   = x * sum_d sigmoid(depthwise3x3(x, kern, dil=d)),  d in (1,3,5)

Mapping (fp8 DoubleRow edition):
  - Depthwise conv = accumulating matmuls with a DIAGONAL stationary
    matrix diag(kern[:,tap]) and a SHIFTED WINDOW of x as the moving
    operand, in the flat padded layout (row stride 69 = 64 data + 5
    shared zero margin, 5 zero rows top/bottom) so any (dy,dx) shift
    with |dy*d|,|dx*d| <= 5 is a pure 1-D offset.
  - The 8 outer taps run as 4 fp8e4m3 DoubleRow matmuls per
    (cb, dilation, window): each DR matmul computes TWO taps in one
    N-cycle pass (2 fp8 elements/cycle), halving PE streaming time.
    Weights = adjacent diag slots (pair stride 128 B); moving operand =
    3-D AP [128, 2, N] whose k-tile stride is the tap shift delta.
  - The center tap must stay fp16 for precision (fp8 here costs ~3x
    error margin): VectorE scalar_tensor_tensor accumulates
    kern_c * x_fp16 into the psum after the DR matmuls (last windows of
    the last cb use an fp16 PE matmul instead to shorten the drain).
  - query and G^T GEMMs also run fp8 DoubleRow (2 resp 18 matmuls).
  - The three dilation psums live in ONE 3-bank psum tile [128, 3, 512]
    so a single strided ACT computes all three sigmoids per window.
  - sums: s0+s1 on VectorE (fp16 2x), +s2 and the final x*w on GpSimd.
  - Output stored fp16 (halves store traffic; adds ~5e-3 abs err on a
    scale-10.6 output, well inside the 2e-2 budget).

Measured error (numpy sim of these exact numerics): rel 0.0097 vs the
2e-2 gate; fp16 baseline was 0.0011.
"""
import numpy as np

C, H, W = 512, 64, 64
P, CB = 128, 4
RS = 69                   # row stride: 64 data + 5 shared zero margin
HEAD = 5                  # guard zeros before row 0 (for dx<0 on top pad row)
VPAD = 5                  # zero rows above/below the image
XLEN = 5120               # per-(channel,cb) flat buffer length
RT = 7                    # image rows per conv/query window
NW = 10                   # 9 windows x 7 rows + 1 window x 1 row = 64 rows
NPAD = 4480               # padded n-range covered by q/xT chunks (35*128)
NCH = 35                  # n-chunks of 128
RATES = (1, 3, 5)
# DR tap pairs: taps (0,1),(2,3),(5,6),(7,8); tap 4 (center) is fp16
PAIRS = ((0, 1), (2, 3), (5, 6), (7, 8))
NCORES = 8

_CACHE = {}


def _flat(r, x):
    # buffer index of image row r (may be in [-5, 69)), column x
    return HEAD + (VPAD + r) * RS + x


def _build_program():
    import concourse.bacc as bacc
    import concourse.bass as bass
    import concourse.mybir as mybir
    from concourse.tile import TileContext

    dt = mybir.dt
    AF = mybir.ActivationFunctionType
    ALU = mybir.AluOpType
    DR = mybir.MatmulPerfMode.DoubleRow
    f32, f16, f8 = dt.float32, dt.float16, dt.float8e4

    nc = bacc.Bacc()
    x8_d = nc.declare_dram_parameter("x8", [C, XLEN], f8, isOutput=False)
    xf_d = nc.declare_dram_parameter("xf", [C, XLEN], f16, isOutput=False)
    xT8_d = nc.declare_dram_parameter("xT8", [NPAD, C], f8, isOutput=False)
    wkT_d = nc.declare_dram_parameter("wkT", [C, C], f16, isOutput=False)
    wq8T_d = nc.declare_dram_parameter("wq8T", [C, 16], f8, isOutput=False)
    bk_d = nc.declare_dram_parameter("bk", [C], f32, isOutput=False)
    id9h_d = nc.declare_dram_parameter("id9h", [9, 9], f16, isOutput=False)
    id8_d = nc.declare_dram_parameter("id8", [P, P], f8, isOutput=False)
    out_d = nc.declare_dram_parameter("out", [C, H, W], f16, isOutput=True)

    def winsize(w):
        return (RT * RS) if w < NW - 1 else RS  # 483 or 69

    def nrows(w):
        return RT if w < NW - 1 else 1

    def pair_ap(win, delta):
        # [P, N] window -> [P, 2, N]: second k-tile shifted by delta elems
        return bass.AP(tensor=win.tensor, offset=win.offset,
                       ap=[list(win.ap[0]), [delta, 2], list(win.ap[-1])])

    with TileContext(nc) as tc:
        with (
            tc.tile_pool(name="const", bufs=1) as cpool,
            tc.tile_pool(name="diagp", bufs=2) as dpool,
            tc.tile_pool(name="sigp", bufs=4) as sigp,
            tc.tile_pool(name="tmpp", bufs=3) as tmpp,
            tc.tile_pool(name="outp", bufs=3) as opool,
        ):
            psA = tc.alloc_tile_pool(name="psA", bufs=5, space="PSUM")
            x8 = cpool.tile([P, CB, XLEN], f8)
            xf = cpool.tile([P, CB, XLEN], f16)
            xT8 = cpool.tile([P, NCH, C], f8)
            wkT = cpool.tile([P, CB, C], f16)
            wq8T = cpool.tile([P, CB, 16], f8)
            bk = cpool.tile([P, CB], f32)
            id9h = cpool.tile([9, 9], f16)
            id8 = cpool.tile([P, P], f8)
            # query padded to 32 partitions (rows 9..31 never written/read)
            query = cpool.tile([32, NPAD], f16)
            ssum = cpool.tile([9, 1], f32)
            rinv = cpool.tile([9, 1], f32)
            qT8 = cpool.tile([P, NCH, 32], f8)
            gs = cpool.tile([9, C], f16)
            G = cpool.tile([P, CB, 9], f16)
            kern = cpool.tile([P, CB, 9], f32)

            # ---- input DMAs spread over the two HW-DGE queues (sync,
            # scalar) so loads stream in parallel (the gpsimd queue is
            # software-DGE and ~4x slower).  Per queue the issue order is
            # by need-time: small weights, x8 (query+conv), xT8 (G^T),
            # wkT (kern GEMM), xf (center taps + final mul). ----
            nc.scalar.dma_start(out=id8[:], in_=id8_d[:])
            nc.scalar.dma_start(out=wq8T[:], in_=wq8T_d[:].rearrange(
                "(cb p) t -> p cb t", p=P))
            nc.scalar.dma_start(out=id9h[:], in_=id9h_d[:])
            nc.scalar.dma_start(
                out=bk[:], in_=bk_d[:].rearrange("(cb p) -> p cb", p=P))
            q0, q1 = nc.sync, nc.scalar
            for h in range(2):
                a, b = (0, 2560) if h == 0 else (2560, XLEN)
                for cb in range(CB):
                    (q0 if cb % 2 == 0 else q1).dma_start(
                        out=x8[:, cb, a:b], in_=x8_d[cb * P:(cb + 1) * P, a:b])
            for h in range(4):
                a, b = h * 9, min((h + 1) * 9, NCH)
                (q0 if h % 2 == 0 else q1).dma_start(
                    out=xT8[:, a:b],
                    in_=xT8_d[a * P:b * P].rearrange("(n p) c -> p n c", p=P))
            nc.scalar.dma_start(
                out=wkT[:], in_=wkT_d[:].rearrange("(cb p) o -> p cb o", p=P))
            for cb in range(CB):
                (q0 if cb % 2 == 0 else q1).dma_start(
                    out=xf[:, cb, :], in_=xf_d[cb * P:(cb + 1) * P, :])

            # tail of the padded n-range is not written by any query window
            qwritten = (NW - 1) * RT * RS + RS  # 4416
            nc.vector.memset(query[0:9, qwritten:NPAD], 0.0)

            # ---- PE warmup: ~8us of dummy matmuls on a memset tile (no
            # DMA dependency - starts immediately) so the HAM clock-gate
            # opens before the first real matmul ----
            wz = cpool.tile([P, P], f8)
            nc.vector.memset(wz[:], 0.0)
            pw = psA.tile([P, P], f32, tag="ps")
            for i in range(80):
                nc.tensor.matmul(pw[:], lhsT=wz[:], rhs=wz[:],
                                 start=(i == 0), stop=(i == 79))

            # ---- query = Wq @ x as fp8 DoubleRow (cb pairs), exp fused
            # into the PSUM drain.  No max subtraction: query ~ N(0,1), so
            # exp(query) stays far below the fp16 ceiling.  The per-window
            # e-sum rides along as the ACT's accum_out; the zero margins
            # (5 cols/row) contribute exactly exp(0)=1 each, subtracted as
            # a constant afterwards (9*35+5 = 320 margin cols total). ----
            esum = cpool.tile([9, NW], f32)
            for w in range(NW):
                N = winsize(w)
                base = _flat(RT * w, 0)
                psq = psA.tile([9, N], f32, tag="ps")
                for kc in range(0, CB, 2):
                    nc.tensor.matmul(
                        psq[:],
                        lhsT=wq8T[:, kc:kc + 2, 0:9],
                        rhs=pair_ap(x8[:, kc, base:base + N], XLEN),
                        start=(kc == 0), stop=(kc == 2),
                        perf_mode=DR)
                j0 = RT * w * RS
                nc.scalar.activation(query[0:9, j0:j0 + N], psq[:], AF.Exp,
                                     accum_out=esum[:, w:w + 1])
            nc.vector.tensor_reduce(
                ssum[:], esum[:], axis=mybir.AxisListType.X, op=ALU.add)
            nc.vector.tensor_scalar_add(ssum[:], ssum[:], -320.0)
            nc.vector.reciprocal(rinv[:], ssum[:])

            # ---- transpose e chunks [9,128] -> [128,9], drain to fp8 ----
            for nch in range(NCH):
                pst = psA.tile([P, 9], f16, tag="ps")
                nc.tensor.transpose(
                    pst[:], query[0:9, nch * P:(nch + 1) * P], id9h[:])
                nc.vector.tensor_copy(qT8[:, nch, 0:9], pst[:])

            # ---- G^T[t, ci] = sum_n e[t,n] x[ci,n] fp8 DR over n-chunk
            # pairs (34 chunks paired + 1 normal fp8), scaled by rinv ----
            pgt = psA.tile([9, C], f32, tag="ps")
            for nch in range(0, NCH - 1, 2):
                nc.tensor.matmul(
                    pgt[:], lhsT=qT8[:, nch:nch + 2, 0:9],
                    rhs=xT8[:, nch:nch + 2, :],
                    start=(nch == 0), stop=False, perf_mode=DR)
            nc.tensor.matmul(
                pgt[:], lhsT=qT8[:, NCH - 1, 0:9], rhs=xT8[:, NCH - 1],
                start=False, stop=True)
            nc.vector.tensor_scalar_mul(gs[:], pgt[:], rinv[:])

            # ---- G[ci, t] = gs^T (4 fp16 PE transposes) ----
            for ci in range(CB):
                psx = psA.tile([P, 9], f16, tag="ps")
                nc.tensor.transpose(
                    psx[:], gs[:, ci * P:(ci + 1) * P], id9h[:])
                nc.vector.tensor_copy(G[:, ci], psx[:])

            # ---- kern[c, t] = sum_ci Wk[c,ci] G[ci,t] + bk[c] (fp16) ----
            for co in range(CB):
                psn = psA.tile([P, 9], f32, tag="ps")
                for ci in range(CB):
                    nc.tensor.matmul(
                        psn[:],
                        lhsT=wkT[:, ci, co * P:(co + 1) * P],
                        rhs=G[:, ci],
                        start=(ci == 0), stop=(ci == CB - 1))
                nc.vector.tensor_scalar_add(kern[:, co], psn[:], bk[:, co:co + 1])

            # ---- depthwise convs: fp8 DR tap pairs on PE + fp16 center ----
            # front-phase psum ring is dead from here; hand its banks to
            # the conv's double-buffered 3-bank window tiles
            psA.release()
            psD = tc.alloc_tile_pool(name="psD", bufs=2, space="PSUM")

            TAPS9 = (0, 1, 2, 3, 5, 6, 7, 8, 4)
            diag8s = [dpool.tile([P, 9, P], f8, tag=f"d8_{cb}",
                                 name=f"diag8_{cb}") for cb in range(CB)]
            diag16s = [dpool.tile([P, P], f16, tag=f"d16_{cb}",
                                  name=f"diag16_{cb}") for cb in range(CB)]

            def emit_diag(cb, t, on_vector):
                if t == 4:
                    nc.vector.tensor_scalar_mul(
                        diag16s[cb][:], id8[:], kern[:, cb, 4:5])
                elif on_vector:
                    nc.vector.tensor_scalar_mul(
                        diag8s[cb][:, t], id8[:], kern[:, cb, t:t + 1])
                else:
                    nc.scalar.mul(
                        diag8s[cb][:, t], id8[:], kern[:, cb, t:t + 1])

            # cb0's diags build at conv start, split Scalar/Vector; later
            # cbs' builds are drip-fed one per window of the previous cb
            for j, t in enumerate(TAPS9):
                emit_diag(0, t, on_vector=(j % 2 == 1))

            def emit_tail(cb, w, pd):
                # sigmoid + sums + final mul + store for a finished window;
                # emitted one window late so no engine queue head-blocks
                # behind the sigmoid
                N = winsize(w)
                nr = nrows(w)
                r0 = RT * w
                st = sigp.tile([P, 3, RT * W], f16, tag="sig")
                nc.scalar.activation(
                    st[:, :, 0:nr * W].rearrange(
                        "p d (r c) -> p d r c", c=W),
                    pd[:, :, 0:N].rearrange(
                        "p d (r c) -> p d r c", c=RS)[:, :, 0:nr, 0:W],
                    AF.Sigmoid)
                t01 = tmpp.tile([P, RT * W], f16, tag="t01")
                w3 = tmpp.tile([P, RT * W], f16, tag="w3")
                nc.vector.tensor_add(
                    t01[:, 0:nr * W], st[:, 0, 0:nr * W], st[:, 1, 0:nr * W])
                nc.gpsimd.tensor_add(
                    w3[:, 0:nr * W], t01[:, 0:nr * W], st[:, 2, 0:nr * W])
                ot = opool.tile([P, RT * W], f16, tag="ot")
                nc.gpsimd.tensor_mul(
                    ot[:, 0:nr * W].rearrange("p (r c) -> p r c", c=W),
                    w3[:, 0:nr * W].rearrange("p (r c) -> p r c", c=W),
                    xf[:, cb, _flat(r0, 0):_flat(r0, 0) + N]
                    .rearrange("p (r c) -> p r c", c=RS)[:, :, 0:W])
                if nr > 1:
                    hr = nr // 2
                    nc.sync.dma_start(
                        out=out_d[cb * P:(cb + 1) * P, r0:r0 + hr, :],
                        in_=ot[:, 0:hr * W].rearrange("p (r c) -> p r c", c=W))
                    nc.sync.dma_start(
                        out=out_d[cb * P:(cb + 1) * P, r0 + hr:r0 + nr, :],
                        in_=ot[:, hr * W:nr * W].rearrange("p (r c) -> p r c", c=W))
                else:
                    nc.sync.dma_start(
                        out=out_d[cb * P:(cb + 1) * P, r0:r0 + nr, :],
                        in_=ot[:, 0:nr * W].rearrange("p (r c) -> p r c", c=W))

            from concourse.tile import add_dep_helper
            prev = None
            last_mm = None
            for cb in range(CB):
                for w in range(NW):
                    N = winsize(w)
                    r0 = RT * w
                    # center taps: di 0,1 on DVE (STT), di 2 as an fp16 PE
                    # matmul -- balances PE vs DVE; the last windows of the
                    # last cb go all-PE to shorten the end-of-kernel drain
                    all_pe = (cb == CB - 1 and w >= NW - 2)
                    pd = psD.tile([P, 3, 512], f32, tag="pd")
                    for di, d in enumerate(RATES):
                        offs = {}
                        for t in range(9):
                            dy, dx = t // 3 - 1, t % 3 - 1
                            offs[t] = _flat(r0 + dy * d, dx * d)
                        pe_center = all_pe or di == 2
                        for pi, (t0, t1) in enumerate(PAIRS):
                            mm = nc.tensor.matmul(
                                pd[:, di, 0:N],
                                lhsT=diag8s[cb][:, t0:t0 + 2, :],
                                rhs=pair_ap(x8[:, cb, offs[t0]:offs[t0] + N],
                                            offs[t1] - offs[t0]),
                                start=(pi == 0),
                                stop=(pi == len(PAIRS) - 1 and not pe_center),
                                perf_mode=DR)
                            if pi == 0 and last_mm is not None:
                                # pin PE queue order to program order so the
                                # psum-bank ring keeps its full 6-group reuse
                                # distance (else the scheduler interleaves
                                # windows and PE stalls on bank recycling)
                                add_dep_helper(mm.ins, last_mm.ins,
                                               reason="strict di-group order")
                        if pe_center:
                            mm = nc.tensor.matmul(
                                pd[:, di, 0:N],
                                lhsT=diag16s[cb][:],
                                rhs=xf[:, cb, offs[4]:offs[4] + N],
                                start=False, stop=True,
                                skip_group_check=True)
                        else:
                            nc.vector.scalar_tensor_tensor(
                                pd[:, di, 0:N],
                                in0=xf[:, cb, offs[4]:offs[4] + N],
                                scalar=kern[:, cb, 4:5],
                                in1=pd[:, di, 0:N],
                                op0=ALU.mult, op1=ALU.add)
                        last_mm = mm
                    # drip-feed next cb's diag builds (one per window)
                    if cb + 1 < CB and w < 9:
                        emit_diag(cb + 1, TAPS9[w], on_vector=False)
                    if prev is not None:
                        emit_tail(*prev)
                    prev = (cb, w, pd)
            emit_tail(*prev)
            psD.release()
    nc.finalize()
    return nc


def _get_program():
    if "nc" not in _CACHE:
        _CACHE["nc"] = _build_program()
    return _CACHE["nc"]


def make_in_maps(x, Wk, bk, Wq, bq=None):
    import ml_dtypes
    f8 = ml_dtypes.float8_e4m3
    x = np.ascontiguousarray(np.asarray(x, dtype=np.float32))
    B = x.shape[0]
    assert B == NCORES and x.shape[1:] == (C, H, W)
    xf = np.zeros((B, C, XLEN), dtype=np.float16)
    view = xf[:, :, HEAD:HEAD + (H + 2 * VPAD) * RS]
    view = view.reshape(B, C, H + 2 * VPAD, RS)
    view[:, :, VPAD:VPAD + H, 0:W] = x.astype(np.float16)
    x8 = xf.astype(f8)
    NB = _flat(0, 0)
    xT8 = np.ascontiguousarray(np.swapaxes(x8[:, :, NB:NB + NPAD], 1, 2))
    wq8T = np.zeros((C, 16), dtype=f8)
    wq8T[:, 0:9] = np.ascontiguousarray(
        np.asarray(Wq, np.float32).T).astype(f8)
    shared = {
        "wkT": np.ascontiguousarray(np.asarray(Wk, np.float32).T).astype(np.float16),
        "wq8T": wq8T,
        "bk": np.ascontiguousarray(np.asarray(bk, np.float32)),
        "id9h": np.eye(9, dtype=np.float16),
        "id8": np.eye(P, dtype=f8),
    }
    return [dict(shared, xf=np.ascontiguousarray(xf[i]),
                 x8=np.ascontiguousarray(x8[i]), xT8=xT8[i])
            for i in range(B)]


def kernel(x, Wk, bk, Wq, bq):
    from concourse.bass_utils import run_bass_kernel_spmd

    in_maps = make_in_maps(x, Wk, bk, Wq, bq)
    nc = _get_program()
    res = run_bass_kernel_spmd(nc, in_maps, list(range(NCORES))).results
    return np.stack([np.asarray(res[i]["out"], np.float32)
                     for i in range(NCORES)])


# revision 18
# speedup vs baseline: 1.4767x; 1.4767x over previous
"""Trainium2 Bass kernel for nn_CaC_Module (dynamic-kernel dilated depthwise CNN).

Per-sample computation (b=8 sharded 1/core across 8 NeuronCores):
  query = Wq @ x          (1x1 conv, [9, hw])
  q     = softmax(query over hw)          (bq cancels in softmax -> ignored)
  kern  = Wk @ (x @ q^T) + bk             (associativity: avoids the big
                                           key GEMM entirely; bk folds in
                                           because sum_n q = 1)
  out   = x * sum_d sigmoid(depthwise3x3(x, kern, dil=d)),  d in (1,3,5)

Mapping (fp8 DoubleRow edition):
  - Depthwise conv = accumulating matmuls with a DIAGONAL stationary
    matrix diag(kern[:,tap]) and a SHIFTED WINDOW of x as the moving
    operand, in the flat padded layout (row stride 69 = 64 data + 5
    shared zero margin, 5 zero rows top/bottom) so any (dy,dx) shift
    with |dy*d|,|dx*d| <= 5 is a pure 1-D offset.
  - The 8 outer taps run as 4 fp8e4m3 DoubleRow matmuls per
    (cb, dilation, window): each DR matmul computes TWO taps in one
    N-cycle pass (2 fp8 elements/cycle), halving PE streaming time.
    Weights = adjacent diag slots (pair stride 128 B); moving operand =
    3-D AP [128, 2, N] whose k-tile stride is the tap shift delta.
  - The center tap must stay fp16 for precision (fp8 here costs ~3x
    error margin): VectorE scalar_tensor_tensor accumulates
    kern_c * x_fp16 into the psum after the DR matmuls (last windows of
    the last cb use an fp16 PE matmul instead to shorten the drain).
  - query and G^T GEMMs also run fp8 DoubleRow (2 resp 18 matmuls).
  - The three dilation psums live in ONE 3-bank psum tile [128, 3, 512]
    so a single strided ACT computes all three sigmoids per window.
  - sums: s0+s1 on VectorE (fp16 2x), +s2 and the final x*w on GpSimd.
  - Output stored fp16 (halves store traffic; adds ~5e-3 abs err on a
    scale-10.6 output, well inside the 2e-2 budget).

Measured error (numpy sim of these exact numerics): rel 0.0097 vs the
2e-2 gate; fp16 baseline was 0.0011.
"""
import numpy as np

C, H, W = 512, 64, 64
P, CB = 128, 4
RS = 69                   # row stride: 64 data + 5 shared zero margin
HEAD = 5                  # guard zeros before row 0 (for dx<0 on top pad row)
VPAD = 5                  # zero rows above/below the image
XLEN = 5120               # per-(channel,cb) flat buffer length
RT = 7                    # image rows per conv/query window
NW = 10                   # 9 windows x 7 rows + 1 window x 1 row = 64 rows
NPAD = 4480               # padded n-range covered by q/xT chunks (35*128)
NCH = 35                  # n-chunks of 128
RATES = (1, 3, 5)
# DR tap pairs: taps (0,1),(2,3),(5,6),(7,8); tap 4 (center) is fp16
PAIRS = ((0, 1), (2, 3), (5, 6), (7, 8))
NCORES = 8

_CACHE = {}


def _flat(r, x):
    # buffer index of image row r (may be in [-5, 69)), column x
    return HEAD + (VPAD + r) * RS + x


def _build_program():
    import concourse.bacc as bacc
    import concourse.bass as bass
    import concourse.mybir as mybir
    from concourse.tile import TileContext

    dt = mybir.dt
    AF = mybir.ActivationFunctionType
    ALU = mybir.AluOpType
    DR = mybir.MatmulPerfMode.DoubleRow
    f32, f16, f8 = dt.float32, dt.float16, dt.float8e4

    nc = bacc.Bacc()
    x8_d = nc.declare_dram_parameter("x8", [C, XLEN], f8, isOutput=False)
    xf_d = nc.declare_dram_parameter("xf", [C, XLEN], f16, isOutput=False)
    xT8_d = nc.declare_dram_parameter("xT8", [NPAD, C], f8, isOutput=False)
    wkT_d = nc.declare_dram_parameter("wkT", [C, C], f16, isOutput=False)
    wq8T_d = nc.declare_dram_parameter("wq8T", [C, 16], f8, isOutput=False)
    bk_d = nc.declare_dram_parameter("bk", [C], f32, isOutput=False)
    id9h_d = nc.declare_dram_parameter("id9h", [9, 9], f16, isOutput=False)
    id8_d = nc.declare_dram_parameter("id8", [P, P], f8, isOutput=False)
    out_d = nc.declare_dram_parameter("out", [C, H, W], f16, isOutput=True)

    def winsize(w):
        return (RT * RS) if w < NW - 1 else RS  # 483 or 69

    def nrows(w):
        return RT if w < NW - 1 else 1

    def pair_ap(win, delta):
        # [P, N] window -> [P, 2, N]: second k-tile shifted by delta elems
        return bass.AP(tensor=win.tensor, offset=win.offset,
                       ap=[list(win.ap[0]), [delta, 2], list(win.ap[-1])])

    with TileContext(nc) as tc:
        with (
            tc.tile_pool(name="const", bufs=1) as cpool,
            tc.tile_pool(name="diagp", bufs=2) as dpool,
            tc.tile_pool(name="sigp", bufs=4) as sigp,
            tc.tile_pool(name="tmpp", bufs=3) as tmpp,
            tc.tile_pool(name="outp", bufs=3) as opool,
        ):
            psA = tc.alloc_tile_pool(name="psA", bufs=5, space="PSUM")
            x8 = cpool.tile([P, CB, XLEN], f8)
            xf = cpool.tile([P, CB, XLEN], f16)
            xT8 = cpool.tile([P, NCH, C], f8)
            wkT = cpool.tile([P, CB, C], f16)
            wq8T = cpool.tile([P, CB, 16], f8)
            bk = cpool.tile([P, CB], f32)
            id9h = cpool.tile([9, 9], f16)
            id8 = cpool.tile([P, P], f8)
            # query padded to 32 partitions (rows 9..31 never written/read)
            query = cpool.tile([32, NPAD], f16)
            ssum = cpool.tile([9, 1], f32)
            rinv = cpool.tile([9, 1], f32)
            qT8 = cpool.tile([P, NCH, 32], f8)
            gs = cpool.tile([9, C], f16)
            G = cpool.tile([P, CB, 9], f16)
            kern = cpool.tile([P, CB, 9], f32)

            # ---- input DMAs spread over the two HW-DGE queues (sync,
            # scalar) so loads stream in parallel (the gpsimd queue is
            # software-DGE and ~4x slower).  Per queue the issue order is
            # by need-time: small weights, x8 (query+conv), xT8 (G^T),
            # wkT (kern GEMM), xf (center taps + final mul). ----
            nc.scalar.dma_start(out=id8[:], in_=id8_d[:])
            nc.scalar.dma_start(out=wq8T[:], in_=wq8T_d[:].rearrange(
                "(cb p) t -> p cb t", p=P))
            nc.scalar.dma_start(out=id9h[:], in_=id9h_d[:])
            nc.scalar.dma_start(
                out=bk[:], in_=bk_d[:].rearrange("(cb p) -> p cb", p=P))
            q0, q1 = nc.sync, nc.scalar
            for h in range(2):
                a, b = (0, 2560) if h == 0 else (2560, XLEN)
                for cb in range(CB):
                    (q0 if cb % 2 == 0 else q1).dma_start(
                        out=x8[:, cb, a:b], in_=x8_d[cb * P:(cb + 1) * P, a:b])
            for h in range(4):
                a, b = h * 9, min((h + 1) * 9, NCH)
                (q0 if h % 2 == 0 else q1).dma_start(
                    out=xT8[:, a:b],
                    in_=xT8_d[a * P:b * P].rearrange("(n p) c -> p n c", p=P))
            nc.scalar.dma_start(
                out=wkT[:], in_=wkT_d[:].rearrange("(cb p) o -> p cb o", p=P))
            for cb in range(CB):
                (q0 if cb % 2 == 0 else q1).dma_start(
                    out=xf[:, cb, :], in_=xf_d[cb * P:(cb + 1) * P, :])

            # tail of the padded n-range is not written by any query window
            qwritten = (NW - 1) * RT * RS + RS  # 4416
            nc.vector.memset(query[0:9, qwritten:NPAD], 0.0)

            # ---- PE warmup: ~8us of dummy matmuls on a memset tile (no
            # DMA dependency - starts immediately) so the HAM clock-gate
            # opens before the first real matmul ----
            wz = cpool.tile([P, P], f8)
            nc.vector.memset(wz[:], 0.0)
            pw = psA.tile([P, P], f32, tag="ps")
            for i in range(80):
                nc.tensor.matmul(pw[:], lhsT=wz[:], rhs=wz[:],
                                 start=(i == 0), stop=(i == 79))

            # ---- query = Wq @ x as fp8 DoubleRow (cb pairs), exp fused
            # into the PSUM drain.  No max subtraction: query ~ N(0,1), so
            # exp(query) stays far below the fp16 ceiling.  The per-window
            # e-sum rides along as the ACT's accum_out; the zero margins
            # (5 cols/row) contribute exactly exp(0)=1 each, subtracted as
            # a constant afterwards (9*35+5 = 320 margin cols total). ----
            esum = cpool.tile([9, NW], f32)
            for w in range(NW):
                N = winsize(w)
                base = _flat(RT * w, 0)
                psq = psA.tile([9, N], f32, tag="ps")
                for kc in range(0, CB, 2):
                    nc.tensor.matmul(
                        psq[:],
                        lhsT=wq8T[:, kc:kc + 2, 0:9],
                        rhs=pair_ap(x8[:, kc, base:base + N], XLEN),
                        start=(kc == 0), stop=(kc == 2),
                        perf_mode=DR)
                j0 = RT * w * RS
                nc.scalar.activation(query[0:9, j0:j0 + N], psq[:], AF.Exp,
                                     accum_out=esum[:, w:w + 1])
            nc.vector.tensor_reduce(
                ssum[:], esum[:], axis=mybir.AxisListType.X, op=ALU.add)
            nc.vector.tensor_scalar_add(ssum[:], ssum[:], -320.0)
            nc.vector.reciprocal(rinv[:], ssum[:])

            # ---- transpose e chunks [9,128] -> [128,9], drain to fp8 ----
            for nch in range(NCH):
                pst = psA.tile([P, 9], f16, tag="ps")
                nc.tensor.transpose(
                    pst[:], query[0:9, nch * P:(nch + 1) * P], id9h[:])
                nc.vector.tensor_copy(qT8[:, nch, 0:9], pst[:])

            # ---- G^T[t, ci] = sum_n e[t,n] x[ci,n] fp8 DR over n-chunk
            # pairs (34 chunks paired + 1 normal fp8), scaled by rinv ----
            pgt = psA.tile([9, C], f32, tag="ps")
            for nch in range(0, NCH - 1, 2):
                nc.tensor.matmul(
                    pgt[:], lhsT=qT8[:, nch:nch + 2, 0:9],
                    rhs=xT8[:, nch:nch + 2, :],
                    start=(nch == 0), stop=False, perf_mode=DR)
            nc.tensor.matmul(
                pgt[:], lhsT=qT8[:, NCH - 1, 0:9], rhs=xT8[:, NCH - 1],
                start=False, stop=True)
            nc.vector.tensor_scalar_mul(gs[:], pgt[:], rinv[:])

            # ---- G[ci, t] = gs^T (4 fp16 PE transposes) ----
            for ci in range(CB):
                psx = psA.tile([P, 9], f16, tag="ps")
                nc.tensor.transpose(
                    psx[:], gs[:, ci * P:(ci + 1) * P], id9h[:])
                nc.vector.tensor_copy(G[:, ci], psx[:])

            # ---- kern[c, t] = sum_ci Wk[c,ci] G[ci,t] + bk[c] (fp16) ----
            for co in range(CB):
                psn = psA.tile([P, 9], f32, tag="ps")
                for ci in range(CB):
                    nc.tensor.matmul(
                        psn[:],
                        lhsT=wkT[:, ci, co * P:(co + 1) * P],
                        rhs=G[:, ci],
                        start=(ci == 0), stop=(ci == CB - 1))
                nc.vector.tensor_scalar_add(kern[:, co], psn[:], bk[:, co:co + 1])

            # ---- depthwise convs: fp8 DR tap pairs on PE + fp16 center ----
            # front-phase psum ring is dead from here; hand its banks to
            # the conv's double-buffered 3-bank window tiles
            psA.release()
            psD = tc.alloc_tile_pool(name="psD", bufs=6, space="PSUM")

            TAPS9 = (0, 1, 2, 3, 5, 6, 7, 8, 4)
            diag8s = [dpool.tile([P, 9, P], f8, tag=f"d8_{cb}",
                                 name=f"diag8_{cb}") for cb in range(CB)]
            diag16s = [dpool.tile([P, P], f16, tag=f"d16_{cb}",
                                  name=f"diag16_{cb}") for cb in range(CB)]

            def emit_diag(cb, t, on_vector):
                if t == 4:
                    nc.vector.tensor_scalar_mul(
                        diag16s[cb][:], id8[:], kern[:, cb, 4:5])
                elif on_vector:
                    nc.vector.tensor_scalar_mul(
                        diag8s[cb][:, t], id8[:], kern[:, cb, t:t + 1])
                else:
                    nc.scalar.mul(
                        diag8s[cb][:, t], id8[:], kern[:, cb, t:t + 1])

            # cb0's diags build at conv start, split Scalar/Vector; later
            # cbs' builds are drip-fed one per window of the previous cb
            for j, t in enumerate(TAPS9):
                emit_diag(0, t, on_vector=(j % 2 == 1))

            def emit_tail(cb, w, pds):
                # sigmoid + sums + final mul + store for a finished window;
                # emitted one window late so no engine queue head-blocks
                # behind the sigmoid
                N = winsize(w)
                nr = nrows(w)
                r0 = RT * w
                st = sigp.tile([P, 3, RT * W], f16, tag="sig")
                for di in range(3):
                    nc.scalar.activation(
                        st[:, di, 0:nr * W].rearrange(
                            "p (r c) -> p r c", c=W),
                        pds[di][:, 0:N].rearrange(
                            "p (r c) -> p r c", c=RS)[:, :, 0:W],
                        AF.Sigmoid)
                t01 = tmpp.tile([P, RT * W], f16, tag="t01")
                w3 = tmpp.tile([P, RT * W], f16, tag="w3")
                nc.vector.tensor_add(
                    t01[:, 0:nr * W], st[:, 0, 0:nr * W], st[:, 1, 0:nr * W])
                nc.gpsimd.tensor_add(
                    w3[:, 0:nr * W], t01[:, 0:nr * W], st[:, 2, 0:nr * W])
                ot = opool.tile([P, RT * W], f16, tag="ot")
                nc.gpsimd.tensor_mul(
                    ot[:, 0:nr * W].rearrange("p (r c) -> p r c", c=W),
                    w3[:, 0:nr * W].rearrange("p (r c) -> p r c", c=W),
                    xf[:, cb, _flat(r0, 0):_flat(r0, 0) + N]
                    .rearrange("p (r c) -> p r c", c=RS)[:, :, 0:W])
                if nr > 1:
                    hr = nr // 2
                    nc.sync.dma_start(
                        out=out_d[cb * P:(cb + 1) * P, r0:r0 + hr, :],
                        in_=ot[:, 0:hr * W].rearrange("p (r c) -> p r c", c=W))
                    nc.sync.dma_start(
                        out=out_d[cb * P:(cb + 1) * P, r0 + hr:r0 + nr, :],
                        in_=ot[:, hr * W:nr * W].rearrange("p (r c) -> p r c", c=W))
                else:
                    nc.sync.dma_start(
                        out=out_d[cb * P:(cb + 1) * P, r0:r0 + nr, :],
                        in_=ot[:, 0:nr * W].rearrange("p (r c) -> p r c", c=W))

            prev = None
            for cb in range(CB):
                for w in range(NW):
                    N = winsize(w)
                    r0 = RT * w
                    # center taps: di 0,1 on DVE (STT), di 2 as an fp16 PE
                    # matmul -- balances PE vs DVE; the last windows of the
                    # last cb go all-PE to shorten the end-of-kernel drain
                    all_pe = (cb == CB - 1 and w >= NW - 2)
                    pds = []
                    for di, d in enumerate(RATES):
                        pdi = psD.tile([P, 512], f32, tag="pd",
                                       name=f"pd_{cb}_{w}_{di}")
                        pds.append(pdi)
                        offs = {}
                        for t in range(9):
                            dy, dx = t // 3 - 1, t % 3 - 1
                            offs[t] = _flat(r0 + dy * d, dx * d)
                        pe_center = all_pe or di == 2
                        for pi, (t0, t1) in enumerate(PAIRS):
                            nc.tensor.matmul(
                                pdi[:, 0:N],
                                lhsT=diag8s[cb][:, t0:t0 + 2, :],
                                rhs=pair_ap(x8[:, cb, offs[t0]:offs[t0] + N],
                                            offs[t1] - offs[t0]),
                                start=(pi == 0),
                                stop=(pi == len(PAIRS) - 1 and not pe_center),
                                perf_mode=DR)
                        if pe_center:
                            nc.tensor.matmul(
                                pdi[:, 0:N],
                                lhsT=diag16s[cb][:],
                                rhs=xf[:, cb, offs[4]:offs[4] + N],
                                start=False, stop=True,
                                skip_group_check=True)
                        else:
                            nc.vector.scalar_tensor_tensor(
                                pdi[:, 0:N],
                                in0=xf[:, cb, offs[4]:offs[4] + N],
                                scalar=kern[:, cb, 4:5],
                                in1=pdi[:, 0:N],
                                op0=ALU.mult, op1=ALU.add)
                    # drip-feed next cb's diag builds (one per window)
                    if cb + 1 < CB and w < 9:
                        emit_diag(cb + 1, TAPS9[w], on_vector=False)
                    if prev is not None:
                        emit_tail(*prev)
                    prev = (cb, w, pds)
            emit_tail(*prev)
            psD.release()
    nc.finalize()
    return nc


def _get_program():
    if "nc" not in _CACHE:
        _CACHE["nc"] = _build_program()
    return _CACHE["nc"]


def make_in_maps(x, Wk, bk, Wq, bq=None):
    import ml_dtypes
    f8 = ml_dtypes.float8_e4m3
    x = np.ascontiguousarray(np.asarray(x, dtype=np.float32))
    B = x.shape[0]
    assert B == NCORES and x.shape[1:] == (C, H, W)
    xf = np.zeros((B, C, XLEN), dtype=np.float16)
    view = xf[:, :, HEAD:HEAD + (H + 2 * VPAD) * RS]
    view = view.reshape(B, C, H + 2 * VPAD, RS)
    view[:, :, VPAD:VPAD + H, 0:W] = x.astype(np.float16)
    x8 = xf.astype(f8)
    NB = _flat(0, 0)
    xT8 = np.ascontiguousarray(np.swapaxes(x8[:, :, NB:NB + NPAD], 1, 2))
    wq8T = np.zeros((C, 16), dtype=f8)
    wq8T[:, 0:9] = np.ascontiguousarray(
        np.asarray(Wq, np.float32).T).astype(f8)
    shared = {
        "wkT": np.ascontiguousarray(np.asarray(Wk, np.float32).T).astype(np.float16),
        "wq8T": wq8T,
        "bk": np.ascontiguousarray(np.asarray(bk, np.float32)),
        "id9h": np.eye(9, dtype=np.float16),
        "id8": np.eye(P, dtype=f8),
    }
    return [dict(shared, xf=np.ascontiguousarray(xf[i]),
                 x8=np.ascontiguousarray(x8[i]), xT8=xT8[i])
            for i in range(B)]


def kernel(x, Wk, bk, Wq, bq):
    from concourse.bass_utils import run_bass_kernel_spmd

    in_maps = make_in_maps(x, Wk, bk, Wq, bq)
    nc = _get_program()
    res = run_bass_kernel_spmd(nc, in_maps, list(range(NCORES))).results
    return np.stack([np.asarray(res[i]["out"], np.float32)
                     for i in range(NCORES)])


# revision 23
# speedup vs baseline: 1.5710x; 1.0639x over previous
"""Trainium2 Bass kernel for nn_CaC_Module (dynamic-kernel dilated depthwise CNN).

Per-sample computation (b=8 sharded 1/core across 8 NeuronCores):
  query = Wq @ x          (1x1 conv, [9, hw])
  q     = softmax(query over hw)          (bq cancels in softmax -> ignored)
  kern  = Wk @ (x @ q^T) + bk             (associativity: avoids the big
                                           key GEMM entirely; bk folds in
                                           because sum_n q = 1)
  out   = x * sum_d sigmoid(depthwise3x3(x, kern, dil=d)),  d in (1,3,5)

Mapping (fp8 DoubleRow edition):
  - Depthwise conv = accumulating matmuls with a DIAGONAL stationary
    matrix diag(kern[:,tap]) and a SHIFTED WINDOW of x as the moving
    operand, in the flat padded layout (row stride 69 = 64 data + 5
    shared zero margin, 5 zero rows top/bottom) so any (dy,dx) shift
    with |dy*d|,|dx*d| <= 5 is a pure 1-D offset.
  - The 8 outer taps run as 4 fp8e4m3 DoubleRow matmuls per
    (cb, dilation, window): each DR matmul computes TWO taps in one
    N-cycle pass (2 fp8 elements/cycle), halving PE streaming time.
    Weights = adjacent diag slots (pair stride 128 B); moving operand =
    3-D AP [128, 2, N] whose k-tile stride is the tap shift delta.
  - The center tap must stay fp16 for precision (fp8 here costs ~3x
    error margin): VectorE scalar_tensor_tensor accumulates
    kern_c * x_fp16 into the psum after the DR matmuls (last windows of
    the last cb use an fp16 PE matmul instead to shorten the drain).
  - query and G^T GEMMs also run fp8 DoubleRow (2 resp 18 matmuls).
  - The three dilation psums live in ONE 3-bank psum tile [128, 3, 512]
    so a single strided ACT computes all three sigmoids per window.
  - sums: s0+s1 on VectorE (fp16 2x), +s2 and the final x*w on GpSimd.
  - Output stored fp16 (halves store traffic; adds ~5e-3 abs err on a
    scale-10.6 output, well inside the 2e-2 budget).

Measured error (numpy sim of these exact numerics): rel 0.0097 vs the
2e-2 gate; fp16 baseline was 0.0011.
"""
import numpy as np

C, H, W = 512, 64, 64
P, CB = 128, 4
RS = 69                   # row stride: 64 data + 5 shared zero margin
HEAD = 5                  # guard zeros before row 0 (for dx<0 on top pad row)
VPAD = 5                  # zero rows above/below the image
XLEN = 5120               # per-(channel,cb) flat buffer length
RT = 7                    # image rows per conv/query window
NW = 10                   # 9 windows x 7 rows + 1 window x 1 row = 64 rows
NPAD = 4480               # padded n-range covered by q/xT chunks (35*128)
NCH = 35                  # n-chunks of 128
RATES = (1, 3, 5)
# DR tap pairs: taps (0,1),(2,3),(5,6),(7,8); tap 4 (center) is fp16
PAIRS = ((0, 1), (2, 3), (5, 6), (7, 8))
NCORES = 8

_CACHE = {}


def _flat(r, x):
    # buffer index of image row r (may be in [-5, 69)), column x
    return HEAD + (VPAD + r) * RS + x


def _build_program():
    import concourse.bacc as bacc
    import concourse.bass as bass
    import concourse.mybir as mybir
    from concourse.tile import TileContext

    dt = mybir.dt
    AF = mybir.ActivationFunctionType
    ALU = mybir.AluOpType
    DR = mybir.MatmulPerfMode.DoubleRow
    f32, f16, f8 = dt.float32, dt.float16, dt.float8e4

    nc = bacc.Bacc()
    x8_d = nc.declare_dram_parameter("x8", [C, XLEN], f8, isOutput=False)
    xf_d = nc.declare_dram_parameter("xf", [C, XLEN], f16, isOutput=False)
    xT8_d = nc.declare_dram_parameter("xT8", [NPAD, C], f8, isOutput=False)
    wkT_d = nc.declare_dram_parameter("wkT", [C, C], f16, isOutput=False)
    wq8T_d = nc.declare_dram_parameter("wq8T", [C, 16], f8, isOutput=False)
    bk_d = nc.declare_dram_parameter("bk", [C], f32, isOutput=False)
    id9h_d = nc.declare_dram_parameter("id9h", [9, 9], f16, isOutput=False)
    id8_d = nc.declare_dram_parameter("id8", [P, P], f8, isOutput=False)
    out_d = nc.declare_dram_parameter("out", [C, H, W], f16, isOutput=True)

    def winsize(w):
        return (RT * RS) if w < NW - 1 else RS  # 483 or 69

    def nrows(w):
        return RT if w < NW - 1 else 1

    def pair_ap(win, delta):
        # [P, N] window -> [P, 2, N]: second k-tile shifted by delta elems
        return bass.AP(tensor=win.tensor, offset=win.offset,
                       ap=[list(win.ap[0]), [delta, 2], list(win.ap[-1])])

    with TileContext(nc) as tc:
        with (
            tc.tile_pool(name="const", bufs=1) as cpool,
            tc.tile_pool(name="diagp", bufs=2) as dpool,
            tc.tile_pool(name="sigp", bufs=4) as sigp,
            tc.tile_pool(name="tmpp", bufs=3) as tmpp,
            tc.tile_pool(name="outp", bufs=3) as opool,
        ):
            psA = tc.alloc_tile_pool(name="psA", bufs=5, space="PSUM")
            x8 = cpool.tile([P, CB, XLEN], f8)
            xf = cpool.tile([P, CB, XLEN], f16)
            xT8 = cpool.tile([P, NCH, C], f8)
            wkT = cpool.tile([P, CB, C], f16)
            wq8T = cpool.tile([P, CB, 16], f8)
            bk = cpool.tile([P, CB], f32)
            id9h = cpool.tile([9, 9], f16)
            id8 = cpool.tile([P, P], f8)
            # query padded to 32 partitions (rows 9..31 never written/read)
            query = cpool.tile([32, NPAD], f16)
            ssum = cpool.tile([9, 1], f32)
            rinv = cpool.tile([9, 1], f32)
            qT8 = cpool.tile([P, NCH, 32], f8)
            gs = cpool.tile([9, C], f16)
            G = cpool.tile([P, CB, 9], f16)
            kern = cpool.tile([P, CB, 9], f32)

            # ---- input DMAs spread over the two HW-DGE queues (sync,
            # scalar) so loads stream in parallel (the gpsimd queue is
            # software-DGE and ~4x slower).  Per queue the issue order is
            # by need-time: small weights, x8 (query+conv), xT8 (G^T),
            # wkT (kern GEMM), xf (center taps + final mul). ----
            nc.scalar.dma_start(out=id8[:], in_=id8_d[:])
            nc.scalar.dma_start(out=wq8T[:], in_=wq8T_d[:].rearrange(
                "(cb p) t -> p cb t", p=P))
            nc.scalar.dma_start(out=id9h[:], in_=id9h_d[:])
            nc.scalar.dma_start(
                out=bk[:], in_=bk_d[:].rearrange("(cb p) -> p cb", p=P))
            q0, q1 = nc.sync, nc.scalar
            xbounds = (0, 1725, 3450, XLEN)
            for h in range(3):
                a, b = xbounds[h], xbounds[h + 1]
                for cb in range(CB):
                    (q0 if cb % 2 == 0 else q1).dma_start(
                        out=x8[:, cb, a:b], in_=x8_d[cb * P:(cb + 1) * P, a:b])
            for h in range(4):
                a, b = h * 9, min((h + 1) * 9, NCH)
                (q0 if h % 2 == 0 else q1).dma_start(
                    out=xT8[:, a:b],
                    in_=xT8_d[a * P:b * P].rearrange("(n p) c -> p n c", p=P))
            nc.scalar.dma_start(
                out=wkT[:], in_=wkT_d[:].rearrange("(cb p) o -> p cb o", p=P))
            # xf cb0 first on the sync queue: it gates conv start (center
            # taps + final mul read it); later cbs are needed ~27us apart
            for cb in (0, 2):
                nc.sync.dma_start(
                    out=xf[:, cb, :], in_=xf_d[cb * P:(cb + 1) * P, :])
            for cb in (1, 3):
                nc.scalar.dma_start(
                    out=xf[:, cb, :], in_=xf_d[cb * P:(cb + 1) * P, :])

            # tail of the padded n-range is not written by any query window
            qwritten = (NW - 1) * RT * RS + RS  # 4416
            nc.vector.memset(query[0:9, qwritten:NPAD], 0.0)

            # ---- PE warmup: ~8us of dummy matmuls on a memset tile (no
            # DMA dependency - starts immediately) so the HAM clock-gate
            # opens before the first real matmul ----
            wz = cpool.tile([P, P], f8)
            nc.vector.memset(wz[:], 0.0)
            pw = psA.tile([P, P], f32, tag="ps")
            for i in range(44):
                nc.tensor.matmul(pw[:], lhsT=wz[:], rhs=wz[:],
                                 start=(i == 0), stop=(i == 43))

            # ---- query = Wq @ x as fp8 DoubleRow (cb pairs), exp fused
            # into the PSUM drain.  No max subtraction: query ~ N(0,1), so
            # exp(query) stays far below the fp16 ceiling.  The per-window
            # e-sum rides along as the ACT's accum_out; the zero margins
            # (5 cols/row) contribute exactly exp(0)=1 each, subtracted as
            # a constant afterwards (9*35+5 = 320 margin cols total). ----
            esum = cpool.tile([9, NW], f32)
            for w in range(NW):
                N = winsize(w)
                base = _flat(RT * w, 0)
                psq = psA.tile([9, N], f32, tag="ps")
                for kc in range(0, CB, 2):
                    nc.tensor.matmul(
                        psq[:],
                        lhsT=wq8T[:, kc:kc + 2, 0:9],
                        rhs=pair_ap(x8[:, kc, base:base + N], XLEN),
                        start=(kc == 0), stop=(kc == 2),
                        perf_mode=DR)
                j0 = RT * w * RS
                nc.scalar.activation(query[0:9, j0:j0 + N], psq[:], AF.Exp,
                                     accum_out=esum[:, w:w + 1])
            nc.vector.tensor_reduce(
                ssum[:], esum[:], axis=mybir.AxisListType.X, op=ALU.add)
            nc.vector.tensor_scalar_add(ssum[:], ssum[:], -320.0)
            nc.vector.reciprocal(rinv[:], ssum[:])

            # ---- transpose e chunks [9,128] -> [128,9], drain to fp8 ----
            for nch in range(NCH):
                pst = psA.tile([P, 9], f16, tag="ps")
                nc.tensor.transpose(
                    pst[:], query[0:9, nch * P:(nch + 1) * P], id9h[:])
                nc.vector.tensor_copy(qT8[:, nch, 0:9], pst[:])

            # ---- G^T[t, ci] = sum_n e[t,n] x[ci,n] fp8 DR over n-chunk
            # pairs (34 chunks paired + 1 normal fp8), scaled by rinv ----
            pgt = psA.tile([9, C], f32, tag="ps")
            for nch in range(0, NCH - 1, 2):
                nc.tensor.matmul(
                    pgt[:], lhsT=qT8[:, nch:nch + 2, 0:9],
                    rhs=xT8[:, nch:nch + 2, :],
                    start=(nch == 0), stop=False, perf_mode=DR)
            nc.tensor.matmul(
                pgt[:], lhsT=qT8[:, NCH - 1, 0:9], rhs=xT8[:, NCH - 1],
                start=False, stop=True)
            nc.vector.tensor_scalar_mul(gs[:], pgt[:], rinv[:])

            # ---- G[ci, t] = gs^T (4 fp16 PE transposes) ----
            for ci in range(CB):
                psx = psA.tile([P, 9], f16, tag="ps")
                nc.tensor.transpose(
                    psx[:], gs[:, ci * P:(ci + 1) * P], id9h[:])
                nc.vector.tensor_copy(G[:, ci], psx[:])

            # ---- kern[c, t] = sum_ci Wk[c,ci] G[ci,t] + bk[c] (fp16) ----
            for co in range(CB):
                psn = psA.tile([P, 9], f32, tag="ps")
                for ci in range(CB):
                    nc.tensor.matmul(
                        psn[:],
                        lhsT=wkT[:, ci, co * P:(co + 1) * P],
                        rhs=G[:, ci],
                        start=(ci == 0), stop=(ci == CB - 1))
                nc.vector.tensor_scalar_add(kern[:, co], psn[:], bk[:, co:co + 1])

            # ---- depthwise convs: fp8 DR tap pairs on PE + fp16 center ----
            # front-phase psum ring is dead from here; hand its banks to
            # the conv's double-buffered 3-bank window tiles
            psA.release()
            psD = tc.alloc_tile_pool(name="psD", bufs=6, space="PSUM")

            TAPS9 = (0, 1, 2, 3, 5, 6, 7, 8, 4)
            diag8s = [dpool.tile([P, 9, P], f8, tag=f"d8_{cb}",
                                 name=f"diag8_{cb}") for cb in range(CB)]
            diag16s = [dpool.tile([P, P], f16, tag=f"d16_{cb}",
                                  name=f"diag16_{cb}") for cb in range(CB)]

            def emit_diag(cb, t, on_vector):
                if t == 4:
                    nc.vector.tensor_scalar_mul(
                        diag16s[cb][:], id8[:], kern[:, cb, 4:5])
                elif on_vector:
                    nc.vector.tensor_scalar_mul(
                        diag8s[cb][:, t], id8[:], kern[:, cb, t:t + 1])
                else:
                    nc.scalar.mul(
                        diag8s[cb][:, t], id8[:], kern[:, cb, t:t + 1])

            # cb0's diags build at conv start, split Scalar/Vector; later
            # cbs' builds are drip-fed one per window of the previous cb
            for j, t in enumerate(TAPS9):
                emit_diag(0, t, on_vector=(j % 2 == 1))

            def emit_tail(cb, w, pds, fast=False):
                # sigmoid + sums + final mul + store for a finished window;
                # emitted one window late so no engine queue head-blocks
                # behind the sigmoid.  fast=True (end-of-kernel drain) puts
                # the sums on the quicker DVE instead of GpSimd.
                N = winsize(w)
                nr = nrows(w)
                r0 = RT * w
                st = sigp.tile([P, 3, RT * W], f16, tag="sig")
                for di in range(3):
                    nc.scalar.activation(
                        st[:, di, 0:nr * W].rearrange(
                            "p (r c) -> p r c", c=W),
                        pds[di][:, 0:N].rearrange(
                            "p (r c) -> p r c", c=RS)[:, :, 0:W],
                        AF.Sigmoid)
                t01 = tmpp.tile([P, RT * W], f16, tag="t01")
                w3 = tmpp.tile([P, RT * W], f16, tag="w3")
                eng = nc.vector if fast else nc.gpsimd
                nc.vector.tensor_add(
                    t01[:, 0:nr * W], st[:, 0, 0:nr * W], st[:, 1, 0:nr * W])
                eng.tensor_add(
                    w3[:, 0:nr * W], t01[:, 0:nr * W], st[:, 2, 0:nr * W])
                ot = opool.tile([P, RT * W], f16, tag="ot")
                eng.tensor_mul(
                    ot[:, 0:nr * W].rearrange("p (r c) -> p r c", c=W),
                    w3[:, 0:nr * W].rearrange("p (r c) -> p r c", c=W),
                    xf[:, cb, _flat(r0, 0):_flat(r0, 0) + N]
                    .rearrange("p (r c) -> p r c", c=RS)[:, :, 0:W])
                if nr > 1:
                    hr = nr // 2
                    nc.sync.dma_start(
                        out=out_d[cb * P:(cb + 1) * P, r0:r0 + hr, :],
                        in_=ot[:, 0:hr * W].rearrange("p (r c) -> p r c", c=W))
                    nc.sync.dma_start(
                        out=out_d[cb * P:(cb + 1) * P, r0 + hr:r0 + nr, :],
                        in_=ot[:, hr * W:nr * W].rearrange("p (r c) -> p r c", c=W))
                else:
                    nc.sync.dma_start(
                        out=out_d[cb * P:(cb + 1) * P, r0:r0 + nr, :],
                        in_=ot[:, 0:nr * W].rearrange("p (r c) -> p r c", c=W))

            prev = None
            for cb in range(CB):
                for w in range(NW):
                    N = winsize(w)
                    r0 = RT * w
                    # center taps: di 0,1 on DVE (STT), di 2 as an fp16 PE
                    # matmul -- balances PE vs DVE; the last windows of the
                    # last cb go all-PE to shorten the end-of-kernel drain
                    all_pe = (cb == CB - 1 and w >= NW - 2)
                    pds = []
                    for di, d in enumerate(RATES):
                        pdi = psD.tile([P, 512], f32, tag="pd",
                                       name=f"pd_{cb}_{w}_{di}")
                        pds.append(pdi)
                        offs = {}
                        for t in range(9):
                            dy, dx = t // 3 - 1, t % 3 - 1
                            offs[t] = _flat(r0 + dy * d, dx * d)
                        pe_center = all_pe
                        for pi, (t0, t1) in enumerate(PAIRS):
                            nc.tensor.matmul(
                                pdi[:, 0:N],
                                lhsT=diag8s[cb][:, t0:t0 + 2, :],
                                rhs=pair_ap(x8[:, cb, offs[t0]:offs[t0] + N],
                                            offs[t1] - offs[t0]),
                                start=(pi == 0),
                                stop=(pi == len(PAIRS) - 1 and not pe_center),
                                perf_mode=DR)
                        if pe_center:
                            nc.tensor.matmul(
                                pdi[:, 0:N],
                                lhsT=diag16s[cb][:],
                                rhs=xf[:, cb, offs[4]:offs[4] + N],
                                start=False, stop=True,
                                skip_group_check=True)
                        else:
                            nc.vector.scalar_tensor_tensor(
                                pdi[:, 0:N],
                                in0=xf[:, cb, offs[4]:offs[4] + N],
                                scalar=kern[:, cb, 4:5],
                                in1=pdi[:, 0:N],
                                op0=ALU.mult, op1=ALU.add)
                    # drip-feed next cb's diag builds (one per window)
                    if cb + 1 < CB and w < 9:
                        emit_diag(cb + 1, TAPS9[w], on_vector=False)
                    if prev is not None:
                        emit_tail(*prev, fast=(cb == CB - 1 and w == NW - 1))
                    prev = (cb, w, pds)
            emit_tail(*prev, fast=True)
            psD.release()
    nc.finalize()
    return nc


def _get_program():
    if "nc" not in _CACHE:
        _CACHE["nc"] = _build_program()
    return _CACHE["nc"]


def make_in_maps(x, Wk, bk, Wq, bq=None):
    import ml_dtypes
    f8 = ml_dtypes.float8_e4m3
    x = np.ascontiguousarray(np.asarray(x, dtype=np.float32))
    B = x.shape[0]
    assert B == NCORES and x.shape[1:] == (C, H, W)
    xf = np.zeros((B, C, XLEN), dtype=np.float16)
    view = xf[:, :, HEAD:HEAD + (H + 2 * VPAD) * RS]
    view = view.reshape(B, C, H + 2 * VPAD, RS)
    view[:, :, VPAD:VPAD + H, 0:W] = x.astype(np.float16)
    x8 = xf.astype(f8)
    NB = _flat(0, 0)
    xT8 = np.ascontiguousarray(np.swapaxes(x8[:, :, NB:NB + NPAD], 1, 2))
    wq8T = np.zeros((C, 16), dtype=f8)
    wq8T[:, 0:9] = np.ascontiguousarray(
        np.asarray(Wq, np.float32).T).astype(f8)
    shared = {
        "wkT": np.ascontiguousarray(np.asarray(Wk, np.float32).T).astype(np.float16),
        "wq8T": wq8T,
        "bk": np.ascontiguousarray(np.asarray(bk, np.float32)),
        "id9h": np.eye(9, dtype=np.float16),
        "id8": np.eye(P, dtype=f8),
    }
    return [dict(shared, xf=np.ascontiguousarray(xf[i]),
                 x8=np.ascontiguousarray(x8[i]), xT8=xT8[i])
            for i in range(B)]


def kernel(x, Wk, bk, Wq, bq):
    from concourse.bass_utils import run_bass_kernel_spmd

    in_maps = make_in_maps(x, Wk, bk, Wq, bq)
    nc = _get_program()
    res = run_bass_kernel_spmd(nc, in_maps, list(range(NCORES))).results
    return np.stack([np.asarray(res[i]["out"], np.float32)
                     for i in range(NCORES)])
